# revision 27
# baseline (speedup 1.0000x reference)
"""Trainium2 Bass kernel for CartesianToJacobi.

Math: for each batch b (independent):
    M[i]   = cumsum(m)[i]                     (inclusive)
    S[i,d] = cumsum(m*q)[i,d]                 (inclusive)
    r[0,d]   = S[N-1,d] / M[N-1]              (center of mass)
    r[i,d]   = q[i,d] - S[i-1,d] / M[i-1]     (i >= 1)

Sharding: pure data parallel over batch. B=1024 -> 128 batches per core,
which exactly fills the 128 SBUF partitions; N*D=768 along the free dim.
Cumsums run as native DVE tensor_tensor_scan ops (one per d-plane).
"""

import sys

import numpy as np

sys.path.insert(0, "/opt/trn_rl_repo")

import concourse.bass as bass  # noqa: E402
import concourse.tile as tile  # noqa: E402
from concourse import bacc, bass_utils, mybir  # noqa: E402

B, N, D = 1024, 256, 3
NCORES = 8
BC = B // NCORES  # 128 batches per core == SBUF partition count
F = N * D  # 768


def build_tile_kernel(tc, r_ap, m_ap, q_ap, niter=1, bufs=2):
    nc = tc.nc
    fp32 = mybir.dt.float32
    add = mybir.AluOpType.add
    bypass = mybir.AluOpType.bypass

    with tc.tile_pool(name="main", bufs=bufs) as pool:
        for _ in range(niter):
            qt = pool.tile([BC, F], fp32)
            mt = pool.tile([BC, N], fp32)
            wt = pool.tile([BC, F], fp32)
            St = pool.tile([BC, F], fp32)
            Mt = pool.tile([BC, N], fp32)
            Rt = pool.tile([BC, N], fp32)
            ut = pool.tile([BC, F], fp32)
            rt = pool.tile([BC, F], fp32)

            nc.sync.dma_start(mt[:], m_ap)
            nc.sync.dma_start(qt[:], q_ap)

            # M = inclusive cumsum(m); recipM = 1/M
            nc.vector.tensor_tensor_scan(Mt[:], mt[:], mt[:], 0.0, add, bypass)
            nc.vector.reciprocal_approx_fast(Rt[:], Mt[:])

            # Per-d strided views: v[:, d, :] selects one component plane.
            q3 = qt[:].rearrange("p (n d) -> p d n", d=D)
            w3 = wt[:].rearrange("p (n d) -> p d n", d=D)
            S3 = St[:].rearrange("p (n d) -> p d n", d=D)
            u3 = ut[:].rearrange("p (n d) -> p d n", d=D)

            for d in range(D):
                # w_d = m * q_d
                nc.vector.tensor_mul(w3[:, d, :], q3[:, d, :], mt[:])
            for d in range(D):
                # S_d = cumsum(w_d)
                nc.vector.tensor_tensor_scan(
                    S3[:, d, :], w3[:, d, :], w3[:, d, :], 0.0, add, bypass
                )
            for d in range(D):
                # u_d[i] = S_d[i-1] * recipM[i-1]  (i >= 1)
                nc.vector.tensor_mul(u3[:, d, 1:], S3[:, d, : N - 1], Rt[:, : N - 1])

            # r rows >= 1: r = q - u (cols D..F-1 are contiguous)
            nc.vector.tensor_sub(rt[:, D:], qt[:, D:], ut[:, D:])
            # row 0: r_d = S_d[N-1] * recipM[N-1]
            for d in range(D):
                col = (N - 1) * D + d
                nc.vector.tensor_mul(
                    rt[:, d : d + 1], St[:, col : col + 1], Rt[:, N - 1 : N]
                )

            nc.sync.dma_start(r_ap, rt[:])


def build_tile_kernel_v2(
    tc,
    r_ap,
    m_ap,
    q_ap,
    niter=1,
    bufs=4,
    CH=2,
    use_gpsimd=True,
    ring_split=False,
    single_store=False,
    m_swdge=False,
):
    """Chunked + fused + (optionally) GPSIMD-offloaded version.

    N is split into CH chunks so q loads / r stores overlap compute.
    Elementwise ops are broadcast-fused over the D(=3) component planes;
    the d=2 plane of w/u/sub moves to GPSIMD when use_gpsimd is set.
    """
    nc = tc.nc
    fp32 = mybir.dt.float32
    add = mybir.AluOpType.add
    mult = mybir.AluOpType.mult
    bypass = mybir.AluOpType.bypass
    NCH = N // CH
    FCH = NCH * D

    def d3(ap):
        return ap.rearrange("p (n d) -> p d n", d=D)

    # Two HWDGE rings on TRN2: SP (nc.sync) and ACT (nc.scalar). Splitting
    # loads/stores across them removes per-DMA issue serialization.
    load_eng = tc.nc.sync
    store_eng = tc.nc.scalar if ring_split else tc.nc.sync
    alt_load_eng = tc.nc.scalar if ring_split else tc.nc.sync

    with tc.tile_pool(name="main", bufs=bufs) as pool:
        for _ in range(niter):
            mt = pool.tile([BC, N], fp32)
            Mt = pool.tile([BC, N], fp32)
            Rt = pool.tile([BC, N], fp32)
            r0t = pool.tile([BC, D], fp32)
            rbig = (
                pool.tile([BC, F], fp32, name="rbig", tag="rbig")
                if single_store
                else None
            )

            (tc.nc.gpsimd if m_swdge else load_eng).dma_start(mt[:], m_ap)
            qts = []
            for c in range(CH):
                qt = pool.tile([BC, FCH], fp32, tag="qt")
                # chunk 0 rides the other ring so its transfer overlaps m's
                eng = alt_load_eng if c == 0 else load_eng
                eng.dma_start(qt[:], q_ap[:, c * FCH : (c + 1) * FCH])
                qts.append(qt)

            nc.vector.tensor_tensor_scan(Mt[:], mt[:], mt[:], 0.0, add, bypass)
            nc.vector.reciprocal_approx_fast(Rt[:], Mt[:])

            Sts = []
            for c in range(CH):
                qt = qts[c]
                wt = pool.tile([BC, FCH], fp32, tag="wt")
                St = pool.tile([BC, FCH], fp32, tag="St")
                ut = pool.tile([BC, FCH], fp32, tag="ut")
                if single_store:
                    rt = rbig[:, c * FCH : (c + 1) * FCH]
                else:
                    rt = pool.tile([BC, FCH], fp32, name="rt", tag="rt")[:]
                Sts.append(St)
                q3, w3, S3, u3, r3 = d3(qt[:]), d3(wt[:]), d3(St[:]), d3(ut[:]), d3(rt)

                m_sl = mt[:, c * NCH : (c + 1) * NCH]
                # ---- w = m * q (broadcast m over d)
                if use_gpsimd:
                    m_b2 = m_sl.unsqueeze(1).broadcast_to((BC, 2, NCH))
                    nc.vector.tensor_tensor(w3[:, 0:2, :], q3[:, 0:2, :], m_b2, mult)
                    nc.gpsimd.tensor_tensor(w3[:, 2, :], q3[:, 2, :], m_sl, mult)
                else:
                    m_b3 = m_sl.unsqueeze(1).broadcast_to((BC, D, NCH))
                    nc.vector.tensor_tensor(w3[:, :, :], q3[:, :, :], m_b3, mult)

                # ---- S_d = cumsum(w_d), chained across chunks
                for d in range(D):
                    if c == 0:
                        init = 0.0
                    else:
                        init = d3(Sts[c - 1][:])[:, d, NCH - 1 : NCH]
                    nc.vector.tensor_tensor_scan(
                        S3[:, d, :], w3[:, d, :], w3[:, d, :], init, add, bypass
                    )

                # ---- row 0 (needs only last chunk's scans): issue early
                if c == CH - 1:
                    if single_store:
                        r0o = rbig[:, 0:D].unsqueeze(2)  # [p, D, 1]
                    else:
                        r0o = r0t[:].unsqueeze(2)  # [p, D, 1]
                    s_last = S3[:, :, NCH - 1 : NCH]
                    R_last = (
                        Rt[:, N - 1 : N].unsqueeze(1).broadcast_to((BC, D, 1))
                    )
                    nc.vector.tensor_tensor(r0o, s_last, R_last, mult)
                    if not single_store:
                        store_eng.dma_start(r_ap[:, 0:D], r0t[:])

                # ---- u_d[i] = S_d[i-1] * recipM[i-1]
                if c > 0:
                    # boundary element n = c*NCH uses prev chunk's last S col
                    sprev = d3(Sts[c - 1][:])[:, :, NCH - 1 : NCH]
                    rprev = (
                        Rt[:, c * NCH - 1 : c * NCH]
                        .unsqueeze(1)
                        .broadcast_to((BC, D, 1))
                    )
                    nc.vector.tensor_tensor(u3[:, :, 0:1], sprev, rprev, mult)
                R_main = (
                    Rt[:, c * NCH : (c + 1) * NCH - 1]
                    .unsqueeze(1)
                    .broadcast_to((BC, D, NCH - 1))
                )
                if use_gpsimd:
                    nc.vector.tensor_tensor(
                        u3[:, 0:2, 1:], S3[:, 0:2, : NCH - 1], R_main[:, 0:2, :], mult
                    )
                    nc.gpsimd.tensor_tensor(
                        u3[:, 2, 1:], S3[:, 2, : NCH - 1], R_main[:, 2, :], mult
                    )
                else:
                    nc.vector.tensor_tensor(
                        u3[:, :, 1:], S3[:, :, : NCH - 1], R_main, mult
                    )

                # ---- r = q - u; chunk 0 skips n=0 (row 0 handled separately)
                lo_n = 1 if c == 0 else 0
                if use_gpsimd:
                    nc.vector.tensor_sub(
                        r3[:, 0:2, lo_n:], q3[:, 0:2, lo_n:], u3[:, 0:2, lo_n:]
                    )
                    nc.gpsimd.tensor_sub(
                        r3[:, 2, lo_n:], q3[:, 2, lo_n:], u3[:, 2, lo_n:]
                    )
                else:
                    nc.vector.tensor_sub(
                        r3[:, :, lo_n:], q3[:, :, lo_n:], u3[:, :, lo_n:]
                    )
                if single_store:
                    if c == CH - 1:
                        store_eng.dma_start(r_ap, rbig[:])
                else:
                    lo = lo_n * D
                    # early chunks store on the (now idle) load ring; the
                    # final chunk takes the store ring so it is never queued
                    # behind them
                    eng = store_eng if c == CH - 1 else load_eng
                    eng.dma_start(
                        r_ap[:, c * FCH + lo : (c + 1) * FCH], rt[:, lo:]
                    )


def build_tile_kernel_v3(
    tc,
    r_ap,
    m_ap,
    q_ap,
    niter=1,
    bufs=2,
    splits=(128, 128),
    early_store_act=False,
):
    """Big-tile variant: one [BC, F] tile per tensor; chunked DMA + ops via
    column-range APs. Ops span chunk boundaries (no per-boundary fixups).

    splits: n-counts per chunk (sum == N).
    """
    nc = tc.nc
    fp32 = mybir.dt.float32
    add = mybir.AluOpType.add
    mult = mybir.AluOpType.mult
    bypass = mybir.AluOpType.bypass
    assert sum(splits) == N

    def d3(ap):
        return ap.rearrange("p (n d) -> p d n", d=D)

    with tc.tile_pool(name="main", bufs=bufs) as pool:
        for _ in range(niter):
            mt = pool.tile([BC, N], fp32)
            Mt = pool.tile([BC, N], fp32)
            Rt = pool.tile([BC, N], fp32)
            r0t = pool.tile([BC, D], fp32)
            qt = pool.tile([BC, F], fp32)
            wt = pool.tile([BC, F], fp32)
            St = pool.tile([BC, F], fp32)
            ut = pool.tile([BC, F], fp32)
            rt = pool.tile([BC, F], fp32)
            q3, w3, S3, u3, r3 = (
                d3(qt[:]),
                d3(wt[:]),
                d3(St[:]),
                d3(ut[:]),
                d3(rt[:]),
            )

            nc.sync.dma_start(mt[:], m_ap)
            n0 = 0
            for nn in splits:
                nc.sync.dma_start(
                    qt[:, n0 * D : (n0 + nn) * D],
                    q_ap[:, n0 * D : (n0 + nn) * D],
                )
                n0 += nn

            nc.vector.tensor_tensor_scan(Mt[:], mt[:], mt[:], 0.0, add, bypass)
            nc.vector.reciprocal_approx_fast(Rt[:], Mt[:])

            n0 = 0
            for ci, nn in enumerate(splits):
                n1 = n0 + nn
                m_b = (
                    mt[:, n0:n1].unsqueeze(1).broadcast_to((BC, D, nn))
                )
                nc.vector.tensor_tensor(
                    w3[:, :, n0:n1], q3[:, :, n0:n1], m_b, mult
                )
                for d in range(D):
                    init = 0.0 if ci == 0 else S3[:, d, n0 - 1 : n0]
                    nc.vector.tensor_tensor_scan(
                        S3[:, d, n0:n1], w3[:, d, n0:n1], w3[:, d, n0:n1],
                        init, add, bypass,
                    )
                if ci == len(splits) - 1:
                    # row 0: S_d[N-1] * recipM[N-1] -> tiny patch store on the
                    # ACT HWDGE ring so it never blocks SP-ring chunk stores
                    r0o = r0t[:].unsqueeze(2)
                    s_last = S3[:, :, N - 1 : N]
                    R_last = (
                        Rt[:, N - 1 : N].unsqueeze(1).broadcast_to((BC, D, 1))
                    )
                    nc.vector.tensor_tensor(r0o, s_last, R_last, mult)
                    nc.scalar.dma_start(r_ap[:, 0:D], r0t[:])
                lo = 1 if ci == 0 else n0
                R_b = (
                    Rt[:, lo - 1 : n1 - 1]
                    .unsqueeze(1)
                    .broadcast_to((BC, D, n1 - lo))
                )
                nc.vector.tensor_tensor(
                    u3[:, :, lo:n1], S3[:, :, lo - 1 : n1 - 1], R_b, mult
                )
                nc.vector.tensor_sub(
                    r3[:, :, lo:n1], q3[:, :, lo:n1], u3[:, :, lo:n1]
                )
                # store this chunk (chunk 0 skips n=0; covered by r0t patch)
                last = ci == len(splits) - 1
                eng = nc.scalar if (early_store_act and not last) else nc.sync
                eng.dma_start(r_ap[:, lo * D : n1 * D], rt[:, lo * D : n1 * D])
                n0 = n1


def build_tile_kernel_v4(
    tc,
    r_ap,
    mq_ap,
    niter=1,
    bufs=2,
    splits=(64, 192),
    load_splits=None,
    early_store_act=False,
):
    """Packed-input variant: one DRAM input [BC, N + F] = [m | q-interleaved].

    The first load DMA carries m plus the first q chunk, so cumsum(m) and the
    first w-mul start as early as possible after a single DMA completion.

    splits: n-counts per compute chunk.
    load_splits: n-counts per load DMA (first one also carries m);
                 defaults to splits.
    """
    nc = tc.nc
    fp32 = mybir.dt.float32
    add = mybir.AluOpType.add
    mult = mybir.AluOpType.mult
    bypass = mybir.AluOpType.bypass
    assert sum(splits) == N
    if load_splits is None:
        load_splits = splits
    assert sum(load_splits) == N

    def d3(ap):
        return ap.rearrange("p (n d) -> p d n", d=D)

    with tc.tile_pool(name="main", bufs=bufs) as pool:
        for _ in range(niter):
            Mt = pool.tile([BC, N], fp32)
            Rt = pool.tile([BC, N], fp32)
            r0t = pool.tile([BC, D], fp32)
            data = pool.tile([BC, N + F], fp32)
            St = pool.tile([BC, F], fp32)
            ut = pool.tile([BC, F], fp32)
            rt = pool.tile([BC, F], fp32)
            mt = data[:, 0:N]
            q3 = d3(data[:, N : N + F])
            S3, u3, r3 = d3(St[:]), d3(ut[:]), d3(rt[:])
            wt = pool.tile([BC, F], fp32)
            w3 = d3(wt[:])

            # loads: first DMA = m + first q chunk; rest per load split
            n0 = 0
            for li, nn in enumerate(load_splits):
                lo_col = 0 if li == 0 else N + n0 * D
                hi_col = N + (n0 + nn) * D
                nc.sync.dma_start(
                    data[:, lo_col:hi_col], mq_ap[:, lo_col:hi_col]
                )
                n0 += nn

            nc.vector.tensor_tensor_scan(Mt[:], mt, mt, 0.0, add, bypass)

            n0 = 0
            for ci, nn in enumerate(splits):
                n1 = n0 + nn
                m_b = mt[:, n0:n1].unsqueeze(1).broadcast_to((BC, D, nn))
                nc.vector.tensor_tensor(
                    w3[:, :, n0:n1], q3[:, :, n0:n1], m_b, mult
                )
                for d in range(D):
                    init = 0.0 if ci == 0 else S3[:, d, n0 - 1 : n0]
                    nc.vector.tensor_tensor_scan(
                        S3[:, d, n0:n1], w3[:, d, n0:n1], w3[:, d, n0:n1],
                        init, add, bypass,
                    )
                if ci == 0:
                    # reciprocal off the pre-q critical path, before first u
                    nc.vector.reciprocal_approx_fast(Rt[:], Mt[:])
                if ci == len(splits) - 1:
                    r0o = r0t[:].unsqueeze(2)
                    s_last = S3[:, :, N - 1 : N]
                    R_last = (
                        Rt[:, N - 1 : N].unsqueeze(1).broadcast_to((BC, D, 1))
                    )
                    nc.vector.tensor_tensor(r0o, s_last, R_last, mult)
                    nc.scalar.dma_start(r_ap[:, 0:D], r0t[:])
                lo = 1 if ci == 0 else n0
                R_b = (
                    Rt[:, lo - 1 : n1 - 1]
                    .unsqueeze(1)
                    .broadcast_to((BC, D, n1 - lo))
                )
                nc.vector.tensor_tensor(
                    u3[:, :, lo:n1], S3[:, :, lo - 1 : n1 - 1], R_b, mult
                )
                nc.vector.tensor_sub(
                    r3[:, :, lo:n1], q3[:, :, lo:n1], u3[:, :, lo:n1]
                )
                # non-final stores can ride the ACT ring so the final store
                # never queues behind them on SP
                last = ci == len(splits) - 1
                eng = nc.scalar if (early_store_act and not last) else nc.sync
                eng.dma_start(r_ap[:, lo * D : n1 * D], rt[:, lo * D : n1 * D])
                n0 = n1


_CACHE = {}


VARIANT = "v3"
VARIANT_KW = dict(splits=(96, 160), early_store_act=True, bufs=2)


def build_program(niter=1, variant="v2", **kw):
    nc = bacc.Bacc(
        "TRN2", target_bir_lowering=False, debug=False, enable_asserts=False
    )
    r_t = nc.dram_tensor("r_out", (BC, F), mybir.dt.float32, kind="ExternalOutput")
    if variant == "v4":
        mq_t = nc.dram_tensor(
            "mq_in", (BC, N + F), mybir.dt.float32, kind="ExternalInput"
        )
        with tile.TileContext(nc) as tc:
            build_tile_kernel_v4(tc, r_t.ap(), mq_t.ap(), niter=niter, **kw)
    else:
        m_t = nc.dram_tensor("m_in", (BC, N), mybir.dt.float32, kind="ExternalInput")
        q_t = nc.dram_tensor("q_in", (BC, F), mybir.dt.float32, kind="ExternalInput")
        builder = {
            "v1": build_tile_kernel,
            "v2": build_tile_kernel_v2,
            "v3": build_tile_kernel_v3,
        }[variant]
        with tile.TileContext(nc) as tc:
            builder(tc, r_t.ap(), m_t.ap(), q_t.ap(), niter=niter, **kw)
    nc.compile()
    return nc


def make_in_maps(m, q, variant):
    in_maps = []
    for c in range(NCORES):
        sl = slice(c * BC, (c + 1) * BC)
        if variant == "v4":
            mq = np.concatenate([m[sl], q[sl].reshape(BC, F)], axis=1)
            in_maps.append({"mq_in": np.ascontiguousarray(mq)})
        else:
            in_maps.append({"m_in": m[sl], "q_in": q[sl].reshape(BC, F)})
    return in_maps


def _get_compiled():
    if "nc" not in _CACHE:
        _CACHE["nc"] = build_program(niter=1, variant=VARIANT, **VARIANT_KW)
    return _CACHE["nc"]


def kernel(m: np.ndarray, q: np.ndarray, **run_kwargs):
    m = np.ascontiguousarray(np.asarray(m, dtype=np.float32))
    q = np.ascontiguousarray(np.asarray(q, dtype=np.float32))
    assert m.shape == (B, N) and q.shape == (B, N, D), (m.shape, q.shape)

    nc = _get_compiled()
    in_maps = make_in_maps(m, q, VARIANT)
    res = bass_utils.run_bass_kernel_spmd(
        nc, in_maps, core_ids=list(range(NCORES)), **run_kwargs
    )
    out = np.concatenate(
        [res.results[c]["r_out"].reshape(BC, N, D) for c in range(NCORES)], axis=0
    )
    if run_kwargs:
        _CACHE["last_results"] = res
    return out


# revision 29
# speedup vs baseline: 1.0258x; 1.0258x over previous
"""Trainium2 Bass kernel for CartesianToJacobi.

Math: for each batch b (independent):
    M[i]   = cumsum(m)[i]                     (inclusive)
    S[i,d] = cumsum(m*q)[i,d]                 (inclusive)
    r[0,d]   = S[N-1,d] / M[N-1]              (center of mass)
    r[i,d]   = q[i,d] - S[i-1,d] / M[i-1]     (i >= 1)

Sharding: pure data parallel over batch. B=1024 -> 128 batches per core,
which exactly fills the 128 SBUF partitions; N*D=768 along the free dim.
Cumsums run as native DVE tensor_tensor_scan ops (one per d-plane).
"""

import sys

import numpy as np

sys.path.insert(0, "/opt/trn_rl_repo")

import concourse.bass as bass  # noqa: E402
import concourse.tile as tile  # noqa: E402
from concourse import bacc, bass_utils, mybir  # noqa: E402

B, N, D = 1024, 256, 3
NCORES = 8
BC = B // NCORES  # 128 batches per core == SBUF partition count
F = N * D  # 768


def build_tile_kernel(tc, r_ap, m_ap, q_ap, niter=1, bufs=2):
    nc = tc.nc
    fp32 = mybir.dt.float32
    add = mybir.AluOpType.add
    bypass = mybir.AluOpType.bypass

    with tc.tile_pool(name="main", bufs=bufs) as pool:
        for _ in range(niter):
            qt = pool.tile([BC, F], fp32)
            mt = pool.tile([BC, N], fp32)
            wt = pool.tile([BC, F], fp32)
            St = pool.tile([BC, F], fp32)
            Mt = pool.tile([BC, N], fp32)
            Rt = pool.tile([BC, N], fp32)
            ut = pool.tile([BC, F], fp32)
            rt = pool.tile([BC, F], fp32)

            nc.sync.dma_start(mt[:], m_ap)
            nc.sync.dma_start(qt[:], q_ap)

            # M = inclusive cumsum(m); recipM = 1/M
            nc.vector.tensor_tensor_scan(Mt[:], mt[:], mt[:], 0.0, add, bypass)
            nc.vector.reciprocal_approx_fast(Rt[:], Mt[:])

            # Per-d strided views: v[:, d, :] selects one component plane.
            q3 = qt[:].rearrange("p (n d) -> p d n", d=D)
            w3 = wt[:].rearrange("p (n d) -> p d n", d=D)
            S3 = St[:].rearrange("p (n d) -> p d n", d=D)
            u3 = ut[:].rearrange("p (n d) -> p d n", d=D)

            for d in range(D):
                # w_d = m * q_d
                nc.vector.tensor_mul(w3[:, d, :], q3[:, d, :], mt[:])
            for d in range(D):
                # S_d = cumsum(w_d)
                nc.vector.tensor_tensor_scan(
                    S3[:, d, :], w3[:, d, :], w3[:, d, :], 0.0, add, bypass
                )
            for d in range(D):
                # u_d[i] = S_d[i-1] * recipM[i-1]  (i >= 1)
                nc.vector.tensor_mul(u3[:, d, 1:], S3[:, d, : N - 1], Rt[:, : N - 1])

            # r rows >= 1: r = q - u (cols D..F-1 are contiguous)
            nc.vector.tensor_sub(rt[:, D:], qt[:, D:], ut[:, D:])
            # row 0: r_d = S_d[N-1] * recipM[N-1]
            for d in range(D):
                col = (N - 1) * D + d
                nc.vector.tensor_mul(
                    rt[:, d : d + 1], St[:, col : col + 1], Rt[:, N - 1 : N]
                )

            nc.sync.dma_start(r_ap, rt[:])


def build_tile_kernel_v2(
    tc,
    r_ap,
    m_ap,
    q_ap,
    niter=1,
    bufs=4,
    CH=2,
    use_gpsimd=True,
    ring_split=False,
    single_store=False,
    m_swdge=False,
):
    """Chunked + fused + (optionally) GPSIMD-offloaded version.

    N is split into CH chunks so q loads / r stores overlap compute.
    Elementwise ops are broadcast-fused over the D(=3) component planes;
    the d=2 plane of w/u/sub moves to GPSIMD when use_gpsimd is set.
    """
    nc = tc.nc
    fp32 = mybir.dt.float32
    add = mybir.AluOpType.add
    mult = mybir.AluOpType.mult
    bypass = mybir.AluOpType.bypass
    NCH = N // CH
    FCH = NCH * D

    def d3(ap):
        return ap.rearrange("p (n d) -> p d n", d=D)

    # Two HWDGE rings on TRN2: SP (nc.sync) and ACT (nc.scalar). Splitting
    # loads/stores across them removes per-DMA issue serialization.
    load_eng = tc.nc.sync
    store_eng = tc.nc.scalar if ring_split else tc.nc.sync
    alt_load_eng = tc.nc.scalar if ring_split else tc.nc.sync

    with tc.tile_pool(name="main", bufs=bufs) as pool:
        for _ in range(niter):
            mt = pool.tile([BC, N], fp32)
            Mt = pool.tile([BC, N], fp32)
            Rt = pool.tile([BC, N], fp32)
            r0t = pool.tile([BC, D], fp32)
            rbig = (
                pool.tile([BC, F], fp32, name="rbig", tag="rbig")
                if single_store
                else None
            )

            (tc.nc.gpsimd if m_swdge else load_eng).dma_start(mt[:], m_ap)
            qts = []
            for c in range(CH):
                qt = pool.tile([BC, FCH], fp32, tag="qt")
                # chunk 0 rides the other ring so its transfer overlaps m's
                eng = alt_load_eng if c == 0 else load_eng
                eng.dma_start(qt[:], q_ap[:, c * FCH : (c + 1) * FCH])
                qts.append(qt)

            nc.vector.tensor_tensor_scan(Mt[:], mt[:], mt[:], 0.0, add, bypass)
            nc.vector.reciprocal_approx_fast(Rt[:], Mt[:])

            Sts = []
            for c in range(CH):
                qt = qts[c]
                wt = pool.tile([BC, FCH], fp32, tag="wt")
                St = pool.tile([BC, FCH], fp32, tag="St")
                ut = pool.tile([BC, FCH], fp32, tag="ut")
                if single_store:
                    rt = rbig[:, c * FCH : (c + 1) * FCH]
                else:
                    rt = pool.tile([BC, FCH], fp32, name="rt", tag="rt")[:]
                Sts.append(St)
                q3, w3, S3, u3, r3 = d3(qt[:]), d3(wt[:]), d3(St[:]), d3(ut[:]), d3(rt)

                m_sl = mt[:, c * NCH : (c + 1) * NCH]
                # ---- w = m * q (broadcast m over d)
                if use_gpsimd:
                    m_b2 = m_sl.unsqueeze(1).broadcast_to((BC, 2, NCH))
                    nc.vector.tensor_tensor(w3[:, 0:2, :], q3[:, 0:2, :], m_b2, mult)
                    nc.gpsimd.tensor_tensor(w3[:, 2, :], q3[:, 2, :], m_sl, mult)
                else:
                    m_b3 = m_sl.unsqueeze(1).broadcast_to((BC, D, NCH))
                    nc.vector.tensor_tensor(w3[:, :, :], q3[:, :, :], m_b3, mult)

                # ---- S_d = cumsum(w_d), chained across chunks
                for d in range(D):
                    if c == 0:
                        init = 0.0
                    else:
                        init = d3(Sts[c - 1][:])[:, d, NCH - 1 : NCH]
                    nc.vector.tensor_tensor_scan(
                        S3[:, d, :], w3[:, d, :], w3[:, d, :], init, add, bypass
                    )

                # ---- row 0 (needs only last chunk's scans): issue early
                if c == CH - 1:
                    if single_store:
                        r0o = rbig[:, 0:D].unsqueeze(2)  # [p, D, 1]
                    else:
                        r0o = r0t[:].unsqueeze(2)  # [p, D, 1]
                    s_last = S3[:, :, NCH - 1 : NCH]
                    R_last = (
                        Rt[:, N - 1 : N].unsqueeze(1).broadcast_to((BC, D, 1))
                    )
                    nc.vector.tensor_tensor(r0o, s_last, R_last, mult)
                    if not single_store:
                        store_eng.dma_start(r_ap[:, 0:D], r0t[:])

                # ---- u_d[i] = S_d[i-1] * recipM[i-1]
                if c > 0:
                    # boundary element n = c*NCH uses prev chunk's last S col
                    sprev = d3(Sts[c - 1][:])[:, :, NCH - 1 : NCH]
                    rprev = (
                        Rt[:, c * NCH - 1 : c * NCH]
                        .unsqueeze(1)
                        .broadcast_to((BC, D, 1))
                    )
                    nc.vector.tensor_tensor(u3[:, :, 0:1], sprev, rprev, mult)
                R_main = (
                    Rt[:, c * NCH : (c + 1) * NCH - 1]
                    .unsqueeze(1)
                    .broadcast_to((BC, D, NCH - 1))
                )
                if use_gpsimd:
                    nc.vector.tensor_tensor(
                        u3[:, 0:2, 1:], S3[:, 0:2, : NCH - 1], R_main[:, 0:2, :], mult
                    )
                    nc.gpsimd.tensor_tensor(
                        u3[:, 2, 1:], S3[:, 2, : NCH - 1], R_main[:, 2, :], mult
                    )
                else:
                    nc.vector.tensor_tensor(
                        u3[:, :, 1:], S3[:, :, : NCH - 1], R_main, mult
                    )

                # ---- r = q - u; chunk 0 skips n=0 (row 0 handled separately)
                lo_n = 1 if c == 0 else 0
                if use_gpsimd:
                    nc.vector.tensor_sub(
                        r3[:, 0:2, lo_n:], q3[:, 0:2, lo_n:], u3[:, 0:2, lo_n:]
                    )
                    nc.gpsimd.tensor_sub(
                        r3[:, 2, lo_n:], q3[:, 2, lo_n:], u3[:, 2, lo_n:]
                    )
                else:
                    nc.vector.tensor_sub(
                        r3[:, :, lo_n:], q3[:, :, lo_n:], u3[:, :, lo_n:]
                    )
                if single_store:
                    if c == CH - 1:
                        store_eng.dma_start(r_ap, rbig[:])
                else:
                    lo = lo_n * D
                    # early chunks store on the (now idle) load ring; the
                    # final chunk takes the store ring so it is never queued
                    # behind them
                    eng = store_eng if c == CH - 1 else load_eng
                    eng.dma_start(
                        r_ap[:, c * FCH + lo : (c + 1) * FCH], rt[:, lo:]
                    )


def build_tile_kernel_v3(
    tc,
    r_ap,
    m_ap,
    q_ap,
    niter=1,
    bufs=2,
    splits=(128, 128),
    early_store_act=False,
    m_act=False,
    load_splits=None,
):
    """Big-tile variant: one [BC, F] tile per tensor; chunked DMA + ops via
    column-range APs. Ops span chunk boundaries (no per-boundary fixups).

    splits: n-counts per compute/store chunk (sum == N).
    load_splits: n-counts per q-load DMA (defaults to splits).
    m_act: load m on the ACT HWDGE ring so q loads start immediately on SP.
    """
    nc = tc.nc
    fp32 = mybir.dt.float32
    add = mybir.AluOpType.add
    mult = mybir.AluOpType.mult
    bypass = mybir.AluOpType.bypass
    assert sum(splits) == N
    if load_splits is None:
        load_splits = splits
    assert sum(load_splits) == N

    def d3(ap):
        return ap.rearrange("p (n d) -> p d n", d=D)

    with tc.tile_pool(name="main", bufs=bufs) as pool:
        for _ in range(niter):
            mt = pool.tile([BC, N], fp32)
            Mt = pool.tile([BC, N], fp32)
            Rt = pool.tile([BC, N], fp32)
            r0t = pool.tile([BC, D], fp32)
            qt = pool.tile([BC, F], fp32)
            wt = pool.tile([BC, F], fp32)
            St = pool.tile([BC, F], fp32)
            ut = pool.tile([BC, F], fp32)
            rt = pool.tile([BC, F], fp32)
            q3, w3, S3, u3, r3 = (
                d3(qt[:]),
                d3(wt[:]),
                d3(St[:]),
                d3(ut[:]),
                d3(rt[:]),
            )

            (nc.scalar if m_act else nc.sync).dma_start(mt[:], m_ap)
            n0 = 0
            for nn in load_splits:
                nc.sync.dma_start(
                    qt[:, n0 * D : (n0 + nn) * D],
                    q_ap[:, n0 * D : (n0 + nn) * D],
                )
                n0 += nn

            nc.vector.tensor_tensor_scan(Mt[:], mt[:], mt[:], 0.0, add, bypass)
            nc.vector.reciprocal_approx_fast(Rt[:], Mt[:])

            n0 = 0
            for ci, nn in enumerate(splits):
                n1 = n0 + nn
                m_b = (
                    mt[:, n0:n1].unsqueeze(1).broadcast_to((BC, D, nn))
                )
                nc.vector.tensor_tensor(
                    w3[:, :, n0:n1], q3[:, :, n0:n1], m_b, mult
                )
                for d in range(D):
                    init = 0.0 if ci == 0 else S3[:, d, n0 - 1 : n0]
                    nc.vector.tensor_tensor_scan(
                        S3[:, d, n0:n1], w3[:, d, n0:n1], w3[:, d, n0:n1],
                        init, add, bypass,
                    )
                if ci == len(splits) - 1:
                    # row 0: S_d[N-1] * recipM[N-1] -> tiny patch store on the
                    # ACT HWDGE ring so it never blocks SP-ring chunk stores
                    r0o = r0t[:].unsqueeze(2)
                    s_last = S3[:, :, N - 1 : N]
                    R_last = (
                        Rt[:, N - 1 : N].unsqueeze(1).broadcast_to((BC, D, 1))
                    )
                    nc.vector.tensor_tensor(r0o, s_last, R_last, mult)
                    nc.scalar.dma_start(r_ap[:, 0:D], r0t[:])
                lo = 1 if ci == 0 else n0
                R_b = (
                    Rt[:, lo - 1 : n1 - 1]
                    .unsqueeze(1)
                    .broadcast_to((BC, D, n1 - lo))
                )
                nc.vector.tensor_tensor(
                    u3[:, :, lo:n1], S3[:, :, lo - 1 : n1 - 1], R_b, mult
                )
                nc.vector.tensor_sub(
                    r3[:, :, lo:n1], q3[:, :, lo:n1], u3[:, :, lo:n1]
                )
                # store this chunk (chunk 0 skips n=0; covered by r0t patch)
                last = ci == len(splits) - 1
                eng = nc.scalar if (early_store_act and not last) else nc.sync
                eng.dma_start(r_ap[:, lo * D : n1 * D], rt[:, lo * D : n1 * D])
                n0 = n1


def build_tile_kernel_v4(
    tc,
    r_ap,
    mq_ap,
    niter=1,
    bufs=2,
    splits=(64, 192),
    load_splits=None,
    early_store_act=False,
):
    """Packed-input variant: one DRAM input [BC, N + F] = [m | q-interleaved].

    The first load DMA carries m plus the first q chunk, so cumsum(m) and the
    first w-mul start as early as possible after a single DMA completion.

    splits: n-counts per compute chunk.
    load_splits: n-counts per load DMA (first one also carries m);
                 defaults to splits.
    """
    nc = tc.nc
    fp32 = mybir.dt.float32
    add = mybir.AluOpType.add
    mult = mybir.AluOpType.mult
    bypass = mybir.AluOpType.bypass
    assert sum(splits) == N
    if load_splits is None:
        load_splits = splits
    assert sum(load_splits) == N

    def d3(ap):
        return ap.rearrange("p (n d) -> p d n", d=D)

    with tc.tile_pool(name="main", bufs=bufs) as pool:
        for _ in range(niter):
            Mt = pool.tile([BC, N], fp32)
            Rt = pool.tile([BC, N], fp32)
            r0t = pool.tile([BC, D], fp32)
            data = pool.tile([BC, N + F], fp32)
            St = pool.tile([BC, F], fp32)
            ut = pool.tile([BC, F], fp32)
            rt = pool.tile([BC, F], fp32)
            mt = data[:, 0:N]
            q3 = d3(data[:, N : N + F])
            S3, u3, r3 = d3(St[:]), d3(ut[:]), d3(rt[:])
            wt = pool.tile([BC, F], fp32)
            w3 = d3(wt[:])

            # loads: first DMA = m + first q chunk; rest per load split
            n0 = 0
            for li, nn in enumerate(load_splits):
                lo_col = 0 if li == 0 else N + n0 * D
                hi_col = N + (n0 + nn) * D
                nc.sync.dma_start(
                    data[:, lo_col:hi_col], mq_ap[:, lo_col:hi_col]
                )
                n0 += nn

            nc.vector.tensor_tensor_scan(Mt[:], mt, mt, 0.0, add, bypass)

            n0 = 0
            for ci, nn in enumerate(splits):
                n1 = n0 + nn
                m_b = mt[:, n0:n1].unsqueeze(1).broadcast_to((BC, D, nn))
                nc.vector.tensor_tensor(
                    w3[:, :, n0:n1], q3[:, :, n0:n1], m_b, mult
                )
                for d in range(D):
                    init = 0.0 if ci == 0 else S3[:, d, n0 - 1 : n0]
                    nc.vector.tensor_tensor_scan(
                        S3[:, d, n0:n1], w3[:, d, n0:n1], w3[:, d, n0:n1],
                        init, add, bypass,
                    )
                if ci == 0:
                    # reciprocal off the pre-q critical path, before first u
                    nc.vector.reciprocal_approx_fast(Rt[:], Mt[:])
                if ci == len(splits) - 1:
                    r0o = r0t[:].unsqueeze(2)
                    s_last = S3[:, :, N - 1 : N]
                    R_last = (
                        Rt[:, N - 1 : N].unsqueeze(1).broadcast_to((BC, D, 1))
                    )
                    nc.vector.tensor_tensor(r0o, s_last, R_last, mult)
                    nc.scalar.dma_start(r_ap[:, 0:D], r0t[:])
                lo = 1 if ci == 0 else n0
                R_b = (
                    Rt[:, lo - 1 : n1 - 1]
                    .unsqueeze(1)
                    .broadcast_to((BC, D, n1 - lo))
                )
                nc.vector.tensor_tensor(
                    u3[:, :, lo:n1], S3[:, :, lo - 1 : n1 - 1], R_b, mult
                )
                nc.vector.tensor_sub(
                    r3[:, :, lo:n1], q3[:, :, lo:n1], u3[:, :, lo:n1]
                )
                # non-final stores can ride the ACT ring so the final store
                # never queues behind them on SP
                last = ci == len(splits) - 1
                eng = nc.scalar if (early_store_act and not last) else nc.sync
                eng.dma_start(r_ap[:, lo * D : n1 * D], rt[:, lo * D : n1 * D])
                n0 = n1


_CACHE = {}


VARIANT = "v3"
VARIANT_KW = dict(splits=(96, 160), early_store_act=True, bufs=2)


def build_program(niter=1, variant="v2", **kw):
    nc = bacc.Bacc(
        "TRN2", target_bir_lowering=False, debug=False, enable_asserts=False
    )
    r_t = nc.dram_tensor("r_out", (BC, F), mybir.dt.float32, kind="ExternalOutput")
    if variant == "v4":
        mq_t = nc.dram_tensor(
            "mq_in", (BC, N + F), mybir.dt.float32, kind="ExternalInput"
        )
        with tile.TileContext(nc) as tc:
            build_tile_kernel_v4(tc, r_t.ap(), mq_t.ap(), niter=niter, **kw)
    else:
        m_t = nc.dram_tensor("m_in", (BC, N), mybir.dt.float32, kind="ExternalInput")
        q_t = nc.dram_tensor("q_in", (BC, F), mybir.dt.float32, kind="ExternalInput")
        builder = {
            "v1": build_tile_kernel,
            "v2": build_tile_kernel_v2,
            "v3": build_tile_kernel_v3,
        }[variant]
        with tile.TileContext(nc) as tc:
            builder(tc, r_t.ap(), m_t.ap(), q_t.ap(), niter=niter, **kw)
    nc.compile()
    return nc


def make_in_maps(m, q, variant):
    in_maps = []
    for c in range(NCORES):
        sl = slice(c * BC, (c + 1) * BC)
        if variant == "v4":
            mq = np.concatenate([m[sl], q[sl].reshape(BC, F)], axis=1)
            in_maps.append({"mq_in": np.ascontiguousarray(mq)})
        else:
            in_maps.append({"m_in": m[sl], "q_in": q[sl].reshape(BC, F)})
    return in_maps


def _get_compiled():
    if "nc" not in _CACHE:
        _CACHE["nc"] = build_program(niter=1, variant=VARIANT, **VARIANT_KW)
    return _CACHE["nc"]


def kernel(m: np.ndarray, q: np.ndarray, **run_kwargs):
    m = np.ascontiguousarray(np.asarray(m, dtype=np.float32))
    q = np.ascontiguousarray(np.asarray(q, dtype=np.float32))
    assert m.shape == (B, N) and q.shape == (B, N, D), (m.shape, q.shape)

    nc = _get_compiled()
    in_maps = make_in_maps(m, q, VARIANT)
    res = bass_utils.run_bass_kernel_spmd(
        nc, in_maps, core_ids=list(range(NCORES)), **run_kwargs
    )
    out = np.concatenate(
        [res.results[c]["r_out"].reshape(BC, N, D) for c in range(NCORES)], axis=0
    )
    if run_kwargs:
        _CACHE["last_results"] = res
    return out


# revision 32
# speedup vs baseline: 1.0800x; 1.0528x over previous
"""Trainium2 Bass kernel for CartesianToJacobi.

Math: for each batch b (independent):
    M[i]   = cumsum(m)[i]                     (inclusive)
    S[i,d] = cumsum(m*q)[i,d]                 (inclusive)
    r[0,d]   = S[N-1,d] / M[N-1]              (center of mass)
    r[i,d]   = q[i,d] - S[i-1,d] / M[i-1]     (i >= 1)

Sharding: pure data parallel over batch. B=1024 -> 128 batches per core,
which exactly fills the 128 SBUF partitions; N*D=768 along the free dim.
Cumsums run as native DVE tensor_tensor_scan ops (one per d-plane).
"""

import sys

import numpy as np

sys.path.insert(0, "/opt/trn_rl_repo")

import concourse.bass as bass  # noqa: E402
import concourse.tile as tile  # noqa: E402
from concourse import bacc, bass_utils, mybir  # noqa: E402

B, N, D = 1024, 256, 3
NCORES = 8
BC = B // NCORES  # 128 batches per core == SBUF partition count
F = N * D  # 768


def build_tile_kernel(tc, r_ap, m_ap, q_ap, niter=1, bufs=2):
    nc = tc.nc
    fp32 = mybir.dt.float32
    add = mybir.AluOpType.add
    bypass = mybir.AluOpType.bypass

    with tc.tile_pool(name="main", bufs=bufs) as pool:
        for _ in range(niter):
            qt = pool.tile([BC, F], fp32)
            mt = pool.tile([BC, N], fp32)
            wt = pool.tile([BC, F], fp32)
            St = pool.tile([BC, F], fp32)
            Mt = pool.tile([BC, N], fp32)
            Rt = pool.tile([BC, N], fp32)
            ut = pool.tile([BC, F], fp32)
            rt = pool.tile([BC, F], fp32)

            nc.sync.dma_start(mt[:], m_ap)
            nc.sync.dma_start(qt[:], q_ap)

            # M = inclusive cumsum(m); recipM = 1/M
            nc.vector.tensor_tensor_scan(Mt[:], mt[:], mt[:], 0.0, add, bypass)
            nc.vector.reciprocal_approx_fast(Rt[:], Mt[:])

            # Per-d strided views: v[:, d, :] selects one component plane.
            q3 = qt[:].rearrange("p (n d) -> p d n", d=D)
            w3 = wt[:].rearrange("p (n d) -> p d n", d=D)
            S3 = St[:].rearrange("p (n d) -> p d n", d=D)
            u3 = ut[:].rearrange("p (n d) -> p d n", d=D)

            for d in range(D):
                # w_d = m * q_d
                nc.vector.tensor_mul(w3[:, d, :], q3[:, d, :], mt[:])
            for d in range(D):
                # S_d = cumsum(w_d)
                nc.vector.tensor_tensor_scan(
                    S3[:, d, :], w3[:, d, :], w3[:, d, :], 0.0, add, bypass
                )
            for d in range(D):
                # u_d[i] = S_d[i-1] * recipM[i-1]  (i >= 1)
                nc.vector.tensor_mul(u3[:, d, 1:], S3[:, d, : N - 1], Rt[:, : N - 1])

            # r rows >= 1: r = q - u (cols D..F-1 are contiguous)
            nc.vector.tensor_sub(rt[:, D:], qt[:, D:], ut[:, D:])
            # row 0: r_d = S_d[N-1] * recipM[N-1]
            for d in range(D):
                col = (N - 1) * D + d
                nc.vector.tensor_mul(
                    rt[:, d : d + 1], St[:, col : col + 1], Rt[:, N - 1 : N]
                )

            nc.sync.dma_start(r_ap, rt[:])


def build_tile_kernel_v2(
    tc,
    r_ap,
    m_ap,
    q_ap,
    niter=1,
    bufs=4,
    CH=2,
    use_gpsimd=True,
    ring_split=False,
    single_store=False,
    m_swdge=False,
):
    """Chunked + fused + (optionally) GPSIMD-offloaded version.

    N is split into CH chunks so q loads / r stores overlap compute.
    Elementwise ops are broadcast-fused over the D(=3) component planes;
    the d=2 plane of w/u/sub moves to GPSIMD when use_gpsimd is set.
    """
    nc = tc.nc
    fp32 = mybir.dt.float32
    add = mybir.AluOpType.add
    mult = mybir.AluOpType.mult
    bypass = mybir.AluOpType.bypass
    NCH = N // CH
    FCH = NCH * D

    def d3(ap):
        return ap.rearrange("p (n d) -> p d n", d=D)

    # Two HWDGE rings on TRN2: SP (nc.sync) and ACT (nc.scalar). Splitting
    # loads/stores across them removes per-DMA issue serialization.
    load_eng = tc.nc.sync
    store_eng = tc.nc.scalar if ring_split else tc.nc.sync
    alt_load_eng = tc.nc.scalar if ring_split else tc.nc.sync

    with tc.tile_pool(name="main", bufs=bufs) as pool:
        for _ in range(niter):
            mt = pool.tile([BC, N], fp32)
            Mt = pool.tile([BC, N], fp32)
            Rt = pool.tile([BC, N], fp32)
            r0t = pool.tile([BC, D], fp32)
            rbig = (
                pool.tile([BC, F], fp32, name="rbig", tag="rbig")
                if single_store
                else None
            )

            (tc.nc.gpsimd if m_swdge else load_eng).dma_start(mt[:], m_ap)
            qts = []
            for c in range(CH):
                qt = pool.tile([BC, FCH], fp32, tag="qt")
                # chunk 0 rides the other ring so its transfer overlaps m's
                eng = alt_load_eng if c == 0 else load_eng
                eng.dma_start(qt[:], q_ap[:, c * FCH : (c + 1) * FCH])
                qts.append(qt)

            nc.vector.tensor_tensor_scan(Mt[:], mt[:], mt[:], 0.0, add, bypass)
            nc.vector.reciprocal_approx_fast(Rt[:], Mt[:])

            Sts = []
            for c in range(CH):
                qt = qts[c]
                wt = pool.tile([BC, FCH], fp32, tag="wt")
                St = pool.tile([BC, FCH], fp32, tag="St")
                ut = pool.tile([BC, FCH], fp32, tag="ut")
                if single_store:
                    rt = rbig[:, c * FCH : (c + 1) * FCH]
                else:
                    rt = pool.tile([BC, FCH], fp32, name="rt", tag="rt")[:]
                Sts.append(St)
                q3, w3, S3, u3, r3 = d3(qt[:]), d3(wt[:]), d3(St[:]), d3(ut[:]), d3(rt)

                m_sl = mt[:, c * NCH : (c + 1) * NCH]
                # ---- w = m * q (broadcast m over d)
                if use_gpsimd:
                    m_b2 = m_sl.unsqueeze(1).broadcast_to((BC, 2, NCH))
                    nc.vector.tensor_tensor(w3[:, 0:2, :], q3[:, 0:2, :], m_b2, mult)
                    nc.gpsimd.tensor_tensor(w3[:, 2, :], q3[:, 2, :], m_sl, mult)
                else:
                    m_b3 = m_sl.unsqueeze(1).broadcast_to((BC, D, NCH))
                    nc.vector.tensor_tensor(w3[:, :, :], q3[:, :, :], m_b3, mult)

                # ---- S_d = cumsum(w_d), chained across chunks
                for d in range(D):
                    if c == 0:
                        init = 0.0
                    else:
                        init = d3(Sts[c - 1][:])[:, d, NCH - 1 : NCH]
                    nc.vector.tensor_tensor_scan(
                        S3[:, d, :], w3[:, d, :], w3[:, d, :], init, add, bypass
                    )

                # ---- row 0 (needs only last chunk's scans): issue early
                if c == CH - 1:
                    if single_store:
                        r0o = rbig[:, 0:D].unsqueeze(2)  # [p, D, 1]
                    else:
                        r0o = r0t[:].unsqueeze(2)  # [p, D, 1]
                    s_last = S3[:, :, NCH - 1 : NCH]
                    R_last = (
                        Rt[:, N - 1 : N].unsqueeze(1).broadcast_to((BC, D, 1))
                    )
                    nc.vector.tensor_tensor(r0o, s_last, R_last, mult)
                    if not single_store:
                        store_eng.dma_start(r_ap[:, 0:D], r0t[:])

                # ---- u_d[i] = S_d[i-1] * recipM[i-1]
                if c > 0:
                    # boundary element n = c*NCH uses prev chunk's last S col
                    sprev = d3(Sts[c - 1][:])[:, :, NCH - 1 : NCH]
                    rprev = (
                        Rt[:, c * NCH - 1 : c * NCH]
                        .unsqueeze(1)
                        .broadcast_to((BC, D, 1))
                    )
                    nc.vector.tensor_tensor(u3[:, :, 0:1], sprev, rprev, mult)
                R_main = (
                    Rt[:, c * NCH : (c + 1) * NCH - 1]
                    .unsqueeze(1)
                    .broadcast_to((BC, D, NCH - 1))
                )
                if use_gpsimd:
                    nc.vector.tensor_tensor(
                        u3[:, 0:2, 1:], S3[:, 0:2, : NCH - 1], R_main[:, 0:2, :], mult
                    )
                    nc.gpsimd.tensor_tensor(
                        u3[:, 2, 1:], S3[:, 2, : NCH - 1], R_main[:, 2, :], mult
                    )
                else:
                    nc.vector.tensor_tensor(
                        u3[:, :, 1:], S3[:, :, : NCH - 1], R_main, mult
                    )

                # ---- r = q - u; chunk 0 skips n=0 (row 0 handled separately)
                lo_n = 1 if c == 0 else 0
                if use_gpsimd:
                    nc.vector.tensor_sub(
                        r3[:, 0:2, lo_n:], q3[:, 0:2, lo_n:], u3[:, 0:2, lo_n:]
                    )
                    nc.gpsimd.tensor_sub(
                        r3[:, 2, lo_n:], q3[:, 2, lo_n:], u3[:, 2, lo_n:]
                    )
                else:
                    nc.vector.tensor_sub(
                        r3[:, :, lo_n:], q3[:, :, lo_n:], u3[:, :, lo_n:]
                    )
                if single_store:
                    if c == CH - 1:
                        store_eng.dma_start(r_ap, rbig[:])
                else:
                    lo = lo_n * D
                    # early chunks store on the (now idle) load ring; the
                    # final chunk takes the store ring so it is never queued
                    # behind them
                    eng = store_eng if c == CH - 1 else load_eng
                    eng.dma_start(
                        r_ap[:, c * FCH + lo : (c + 1) * FCH], rt[:, lo:]
                    )


def build_tile_kernel_v3(
    tc,
    r_ap,
    m_ap,
    q_ap,
    niter=1,
    bufs=2,
    splits=(128, 128),
    early_store_act=False,
    m_act=False,
    load_splits=None,
    order_chunks=False,
):
    """Big-tile variant: one [BC, F] tile per tensor; chunked DMA + ops via
    column-range APs. Ops span chunk boundaries (no per-boundary fixups).

    splits: n-counts per compute/store chunk (sum == N).
    load_splits: n-counts per q-load DMA (defaults to splits).
    m_act: load m on the ACT HWDGE ring so q loads start immediately on SP.
    """
    nc = tc.nc
    fp32 = mybir.dt.float32
    add = mybir.AluOpType.add
    mult = mybir.AluOpType.mult
    bypass = mybir.AluOpType.bypass
    assert sum(splits) == N
    if load_splits is None:
        load_splits = splits
    assert sum(load_splits) == N

    def d3(ap):
        return ap.rearrange("p (n d) -> p d n", d=D)

    with tc.tile_pool(name="main", bufs=bufs) as pool:
        for _ in range(niter):
            mt = pool.tile([BC, N], fp32)
            Mt = pool.tile([BC, N], fp32)
            Rt = pool.tile([BC, N], fp32)
            r0t = pool.tile([BC, D], fp32)
            qt = pool.tile([BC, F], fp32)
            wt = pool.tile([BC, F], fp32)
            St = pool.tile([BC, F], fp32)
            ut = pool.tile([BC, F], fp32)
            rt = pool.tile([BC, F], fp32)
            q3, w3, S3, u3, r3 = (
                d3(qt[:]),
                d3(wt[:]),
                d3(St[:]),
                d3(ut[:]),
                d3(rt[:]),
            )

            (nc.scalar if m_act else nc.sync).dma_start(mt[:], m_ap)
            n0 = 0
            for nn in load_splits:
                nc.sync.dma_start(
                    qt[:, n0 * D : (n0 + nn) * D],
                    q_ap[:, n0 * D : (n0 + nn) * D],
                )
                n0 += nn

            nc.vector.tensor_tensor_scan(Mt[:], mt[:], mt[:], 0.0, add, bypass)
            nc.vector.reciprocal_approx_fast(Rt[:], Mt[:])

            n0 = 0
            prev_sub = None
            for ci, nn in enumerate(splits):
                n1 = n0 + nn
                m_b = (
                    mt[:, n0:n1].unsqueeze(1).broadcast_to((BC, D, nn))
                )
                w_inst = nc.vector.tensor_tensor(
                    w3[:, :, n0:n1], q3[:, :, n0:n1], m_b, mult
                )
                if order_chunks and prev_sub is not None:
                    # keep the previous chunk's store-feeding sub ahead of
                    # this chunk's work on the DVE (ordering only, same engine)
                    tile.add_dep_helper(
                        w_inst.ins, prev_sub.ins, sync=False,
                        reason="chunk order: finish prev sub before next w",
                    )
                for d in range(D):
                    init = 0.0 if ci == 0 else S3[:, d, n0 - 1 : n0]
                    nc.vector.tensor_tensor_scan(
                        S3[:, d, n0:n1], w3[:, d, n0:n1], w3[:, d, n0:n1],
                        init, add, bypass,
                    )
                if ci == len(splits) - 1:
                    # row 0: S_d[N-1] * recipM[N-1] -> tiny patch store on the
                    # ACT HWDGE ring so it never blocks SP-ring chunk stores
                    r0o = r0t[:].unsqueeze(2)
                    s_last = S3[:, :, N - 1 : N]
                    R_last = (
                        Rt[:, N - 1 : N].unsqueeze(1).broadcast_to((BC, D, 1))
                    )
                    nc.vector.tensor_tensor(r0o, s_last, R_last, mult)
                    nc.scalar.dma_start(r_ap[:, 0:D], r0t[:])
                lo = 1 if ci == 0 else n0
                R_b = (
                    Rt[:, lo - 1 : n1 - 1]
                    .unsqueeze(1)
                    .broadcast_to((BC, D, n1 - lo))
                )
                nc.vector.tensor_tensor(
                    u3[:, :, lo:n1], S3[:, :, lo - 1 : n1 - 1], R_b, mult
                )
                prev_sub = nc.vector.tensor_sub(
                    r3[:, :, lo:n1], q3[:, :, lo:n1], u3[:, :, lo:n1]
                )
                # store this chunk (chunk 0 skips n=0; covered by r0t patch)
                last = ci == len(splits) - 1
                eng = nc.scalar if (early_store_act and not last) else nc.sync
                eng.dma_start(r_ap[:, lo * D : n1 * D], rt[:, lo * D : n1 * D])
                n0 = n1


def build_tile_kernel_v4(
    tc,
    r_ap,
    mq_ap,
    niter=1,
    bufs=2,
    splits=(64, 192),
    load_splits=None,
    early_store_act=False,
):
    """Packed-input variant: one DRAM input [BC, N + F] = [m | q-interleaved].

    The first load DMA carries m plus the first q chunk, so cumsum(m) and the
    first w-mul start as early as possible after a single DMA completion.

    splits: n-counts per compute chunk.
    load_splits: n-counts per load DMA (first one also carries m);
                 defaults to splits.
    """
    nc = tc.nc
    fp32 = mybir.dt.float32
    add = mybir.AluOpType.add
    mult = mybir.AluOpType.mult
    bypass = mybir.AluOpType.bypass
    assert sum(splits) == N
    if load_splits is None:
        load_splits = splits
    assert sum(load_splits) == N

    def d3(ap):
        return ap.rearrange("p (n d) -> p d n", d=D)

    with tc.tile_pool(name="main", bufs=bufs) as pool:
        for _ in range(niter):
            Mt = pool.tile([BC, N], fp32)
            Rt = pool.tile([BC, N], fp32)
            r0t = pool.tile([BC, D], fp32)
            data = pool.tile([BC, N + F], fp32)
            St = pool.tile([BC, F], fp32)
            ut = pool.tile([BC, F], fp32)
            rt = pool.tile([BC, F], fp32)
            mt = data[:, 0:N]
            q3 = d3(data[:, N : N + F])
            S3, u3, r3 = d3(St[:]), d3(ut[:]), d3(rt[:])
            wt = pool.tile([BC, F], fp32)
            w3 = d3(wt[:])

            # loads: first DMA = m + first q chunk; rest per load split
            n0 = 0
            for li, nn in enumerate(load_splits):
                lo_col = 0 if li == 0 else N + n0 * D
                hi_col = N + (n0 + nn) * D
                nc.sync.dma_start(
                    data[:, lo_col:hi_col], mq_ap[:, lo_col:hi_col]
                )
                n0 += nn

            nc.vector.tensor_tensor_scan(Mt[:], mt, mt, 0.0, add, bypass)

            n0 = 0
            for ci, nn in enumerate(splits):
                n1 = n0 + nn
                m_b = mt[:, n0:n1].unsqueeze(1).broadcast_to((BC, D, nn))
                nc.vector.tensor_tensor(
                    w3[:, :, n0:n1], q3[:, :, n0:n1], m_b, mult
                )
                for d in range(D):
                    init = 0.0 if ci == 0 else S3[:, d, n0 - 1 : n0]
                    nc.vector.tensor_tensor_scan(
                        S3[:, d, n0:n1], w3[:, d, n0:n1], w3[:, d, n0:n1],
                        init, add, bypass,
                    )
                if ci == 0:
                    # reciprocal off the pre-q critical path, before first u
                    nc.vector.reciprocal_approx_fast(Rt[:], Mt[:])
                if ci == len(splits) - 1:
                    r0o = r0t[:].unsqueeze(2)
                    s_last = S3[:, :, N - 1 : N]
                    R_last = (
                        Rt[:, N - 1 : N].unsqueeze(1).broadcast_to((BC, D, 1))
                    )
                    nc.vector.tensor_tensor(r0o, s_last, R_last, mult)
                    nc.scalar.dma_start(r_ap[:, 0:D], r0t[:])
                lo = 1 if ci == 0 else n0
                R_b = (
                    Rt[:, lo - 1 : n1 - 1]
                    .unsqueeze(1)
                    .broadcast_to((BC, D, n1 - lo))
                )
                nc.vector.tensor_tensor(
                    u3[:, :, lo:n1], S3[:, :, lo - 1 : n1 - 1], R_b, mult
                )
                nc.vector.tensor_sub(
                    r3[:, :, lo:n1], q3[:, :, lo:n1], u3[:, :, lo:n1]
                )
                # non-final stores can ride the ACT ring so the final store
                # never queues behind them on SP
                last = ci == len(splits) - 1
                eng = nc.scalar if (early_store_act and not last) else nc.sync
                eng.dma_start(r_ap[:, lo * D : n1 * D], rt[:, lo * D : n1 * D])
                n0 = n1


_CACHE = {}


VARIANT = "v3"
VARIANT_KW = dict(splits=(96, 160), early_store_act=True, bufs=2)


def build_program(niter=1, variant="v2", **kw):
    nc = bacc.Bacc(
        "TRN2", target_bir_lowering=False, debug=False, enable_asserts=False
    )
    r_t = nc.dram_tensor("r_out", (BC, F), mybir.dt.float32, kind="ExternalOutput")
    if variant == "v4":
        mq_t = nc.dram_tensor(
            "mq_in", (BC, N + F), mybir.dt.float32, kind="ExternalInput"
        )
        with tile.TileContext(nc) as tc:
            build_tile_kernel_v4(tc, r_t.ap(), mq_t.ap(), niter=niter, **kw)
    else:
        m_t = nc.dram_tensor("m_in", (BC, N), mybir.dt.float32, kind="ExternalInput")
        q_t = nc.dram_tensor("q_in", (BC, F), mybir.dt.float32, kind="ExternalInput")
        builder = {
            "v1": build_tile_kernel,
            "v2": build_tile_kernel_v2,
            "v3": build_tile_kernel_v3,
        }[variant]
        with tile.TileContext(nc) as tc:
            builder(tc, r_t.ap(), m_t.ap(), q_t.ap(), niter=niter, **kw)
    nc.compile()
    return nc


def make_in_maps(m, q, variant):
    in_maps = []
    for c in range(NCORES):
        sl = slice(c * BC, (c + 1) * BC)
        if variant == "v4":
            mq = np.concatenate([m[sl], q[sl].reshape(BC, F)], axis=1)
            in_maps.append({"mq_in": np.ascontiguousarray(mq)})
        else:
            in_maps.append({"m_in": m[sl], "q_in": q[sl].reshape(BC, F)})
    return in_maps


def _get_compiled():
    if "nc" not in _CACHE:
        _CACHE["nc"] = build_program(niter=1, variant=VARIANT, **VARIANT_KW)
    return _CACHE["nc"]


def kernel(m: np.ndarray, q: np.ndarray, **run_kwargs):
    m = np.ascontiguousarray(np.asarray(m, dtype=np.float32))
    q = np.ascontiguousarray(np.asarray(q, dtype=np.float32))
    assert m.shape == (B, N) and q.shape == (B, N, D), (m.shape, q.shape)

    nc = _get_compiled()
    in_maps = make_in_maps(m, q, VARIANT)
    res = bass_utils.run_bass_kernel_spmd(
        nc, in_maps, core_ids=list(range(NCORES)), **run_kwargs
    )
    out = np.concatenate(
        [res.results[c]["r_out"].reshape(BC, N, D) for c in range(NCORES)], axis=0
    )
    if run_kwargs:
        _CACHE["last_results"] = res
    return out
